# revision 1
# baseline (speedup 1.0000x reference)
"""Trainium2 Bass kernel: C = triu(A @ B), A/B upper-triangular 4096x4096 fp32.

Strategy (row-parallel over 8 cores, SPMD single program):
  * 32 row-blocks of 128 rows. Core c owns blocks {c, 8+c, 16+c, 24+c}
    ("slot" j = block 8j + c).
  * One uniform schedule for all cores: for column tile q (8 tiles of 512)
    and slot j, accumulate k-tiles k in [8j, 4q+3].  Per-core variation
    lives entirely in the DATA: the host packs A^T tiles per core and
    zero-fills tiles with k < own-block, so padded matmuls contribute
    exact zeros.  Since A and B are both upper-triangular, the lower
    triangle of C comes out exactly 0 - no masking needed.
  * A^T pack (80 tiles of 128x128) is cached in SBUF; B streams once per
    column tile with below-diagonal tiles skipped.
"""

import numpy as np
from contextlib import ExitStack

import concourse.mybir as mybir
import concourse.tile as tile
from concourse import bacc, bass_utils

N = 4096
P = 128
NCORES = 8
NSLOT = 4          # row-block slots per core
NQ = 8             # 512-wide output column tiles
QW = 512
NKT = 32           # 128-wide k tiles
KSTART = [0, 8, 16, 24]            # first k-tile per slot (min over cores)
ANT = [32, 24, 16, 8]              # k-tiles stored per slot
AOFF = [0, 32, 56, 72]             # slot offsets into the A pack
ATOT = 80                          # total packed A tiles per core

# (slot, qtile) pairs the program computes/writes, in emission order
PAIRS = [(j, q) for q in range(NQ) for j in range(NSLOT) if 4 * q + 4 > 8 * j]
NT = len(PAIRS)                    # 20 output tiles of 128x512 per core

# matmul dtype mode: "fp32r" (fast, ~11-bit mantissa), "bf16x3" (hi/lo
# 3-pass split, near-fp32 accuracy), "fp32" (exact, 4x slower PE)
MODE = "fp32r"

# pool buffer counts (double/triple buffering)
BUFS_B = 3
BUFS_O = 4
BUFS_PS = 8

_nc_cache = {}


def build_nc(mode=MODE, rep=1, variant="full"):
    """rep>1 repeats the whole compute (for dispatch-overhead-cancelling
    timing): T_hw ~= (T(rep=R) - T(rep=1)) / (R-1).
    variant: "full" | "nomm" (DMAs only) | "nodma" (matmuls only)."""
    if (mode, rep, variant) in _nc_cache:
        return _nc_cache[(mode, rep, variant)]
    two = 2 if mode == "bf16x3" else 1
    dt_in = {
        "fp32r": mybir.dt.float32r,
        "bf16x3": mybir.dt.bfloat16,
        "fp32": mybir.dt.float32,
    }[mode]

    nc = bacc.Bacc("TRN2", target_bir_lowering=False, debug=False,
                   num_devices=NCORES)
    # partition-major packed layouts (see pack_inputs): per-partition data is
    # contiguous so every DMA is 128 descriptors of large contiguous runs.
    # Apack row = h*P + p(k-within-tile), col = t*P + m  (40KB/partition)
    a_dram = nc.dram_tensor("Apack", [two * P, ATOT * P], dt_in,
                            kind="ExternalInput").ap()
    # B row = (h*NQ + q)*P + p, col = k*QW + n          (8KB runs/partition)
    b_dram = nc.dram_tensor("B", [two * NQ * P, NKT * QW], dt_in,
                            kind="ExternalInput").ap()
    c_dram = nc.dram_tensor("Cout", [NT * P, QW], mybir.dt.float32,
                            kind="ExternalOutput").ap()

    with tile.TileContext(nc) as tc:
        with ExitStack() as ctx:
            apool = ctx.enter_context(tc.tile_pool(name="apool", bufs=1))
            bpool = ctx.enter_context(tc.tile_pool(name="bpool", bufs=BUFS_B))
            opool = ctx.enter_context(tc.tile_pool(name="opool", bufs=BUFS_O))
            pspool = ctx.enter_context(
                tc.tile_pool(name="pspool", bufs=BUFS_PS, space="PSUM"))

            do_bdma = variant in ("full", "nomm", "vbdma")
            do_mm = variant in ("full", "nodma", "vmm")
            do_copy = variant in ("full", "nomm", "nodma", "vcopy")
            do_store = variant in ("full", "nomm", "nodma", "vstore")

            # A load split so early matmuls are gated only by the tiles they
            # read: slot0 k0..7 (feeds q=1/q=0) lands in ~1.5us, the rest
            # overlaps with the B stream.
            a_sb = apool.tile([P, two, ATOT, P], dt_in)
            for t0, t1 in [(0, 8), (8, 32), (32, ATOT)]:
                for h in range(two):
                    nc.sync.dma_start(
                        a_sb[:, h, t0:t1, :],
                        a_dram[h * P:(h + 1) * P, t0 * P:t1 * P].rearrange(
                            "p (t m) -> p t m", m=P))

            # micro variants: per rep emit n tiny ops, skip the main loop
            micro = variant.startswith("vd") or variant in ("vgps8", "vdve8")
            if micro:
                n_ops = (8 if variant in ("vgps8", "vdve8")
                         else int(variant[2:]))
                mpool = ctx.enter_context(tc.tile_pool(name="mp", bufs=16))
                for r in range(rep):
                    for i in range(n_ops):
                        mt = mpool.tile([P, QW], mybir.dt.float32, tag="mt",
                                        name=f"mt_{r}_{i}")
                        if variant == "vdve8":
                            src = a_sb[:, 0, 4 * i:4 * i + 4, :]
                            if dt_in == mybir.dt.float32r:
                                src = src.bitcast(mybir.dt.float32)
                            nc.vector.tensor_copy(
                                mt[:].rearrange("p (a b) -> p a b", a=4),
                                src)
                        elif variant == "vgps8":
                            nc.gpsimd.dma_start(
                                mt[:],
                                b_dram[i * P:(i + 1) * P, 0:QW]
                                .bitcast(mybir.dt.float32))
                        else:
                            nc.sync.dma_start(
                                mt[:],
                                b_dram[i * P:(i + 1) * P, 0:QW]
                                .bitcast(mybir.dt.float32))
            bt_fixed = None
            ot_fixed = None

            def _asrc_f32(j):
                src = a_sb[:, 0, 4 * j:4 * j + 4, :]
                if dt_in == mybir.dt.float32r:
                    src = src.bitcast(mybir.dt.float32)
                return src

            if variant == "vstore":
                ot_fixed = opool.tile([P, QW], mybir.dt.float32,
                                      name="ot_fixed")
                nc.vector.tensor_copy(
                    ot_fixed[:].rearrange("p (a b) -> p a b", a=4),
                    _asrc_f32(0))

            def _bsrc(h, kg, q):
                return b_dram[
                    (h * NQ + q) * P:(h * NQ + q + 1) * P,
                    4 * kg * QW:(4 * kg + 4) * QW,
                ].rearrange("p (ko n) -> p ko n", ko=4)

            def _load_diag_chunk(bt, q):
                # per k-row load only the valid columns [128i, 512) -
                # below-diagonal 128-blocks of B are zero
                for h in range(two):
                    for i in range(4):
                        row = (h * NQ + q) * P
                        col = (4 * q + i) * QW + 128 * i
                        nc.sync.dma_start(
                            bt[:, h, i, 128 * i:],
                            b_dram[row:row + P, col:col + QW - 128 * i])

            # q=0's only chunk (0.6MB) is consumed last (Q_ORDER ends on 0):
            # prefetch it into a dedicated buffer at the start so the tail
            # never waits on DMA
            # (tried: prefetching q=0's chunk at the head — model-worse by
            # 1.8us, the DMA stream is saturated so early bytes displace
            # the critical sequence)
            bt_q0 = None

            # q order: q=1 first (ready after the small A-head load), then
            # heaviest-to-lightest so the schedule drains into the tiny q=0
            # tail (4 matmuls + 1 copy + 1 store). Model-swept optimum.
            Q_ORDER = globals().get("_Q_ORDER_OVERRIDE") or \
                [1, 7, 6, 5, 4, 3, 2, 0]
            for _r, q in ([] if micro else
                          [(r, q) for r in range(rep) for q in Q_ORDER]):
                act = [j for j in range(NSLOT) if 4 * q + 4 > 8 * j]
                psums = {
                    j: pspool.tile([P, QW], mybir.dt.float32, tag="ps",
                                   name=f"ps_{_r}_{q}_{j}")
                    for j in act
                } if do_mm else {}
                kend = 4 * q + 3
                for kg in range(q + 1):
                    if do_mm and not do_bdma:
                        if bt_fixed is None:
                            bt_fixed = bpool.tile([P, two, 4, QW], dt_in,
                                                  tag="bt", name="bt_fixed")
                            for h in range(two):
                                nc.sync.dma_start(bt_fixed[:, h],
                                                  _bsrc(h, 0, 0))
                        bt = bt_fixed
                    elif do_bdma or variant == "vmin":
                        if variant == "vmin" and kg > 0:
                            continue
                        if bt_q0 is not None and q == 0:
                            bt = bt_q0
                        else:
                            bt = bpool.tile([P, two, 4, QW], dt_in,
                                            tag="bt")
                            if kg == q:
                                _load_diag_chunk(bt, q)
                            else:
                                for h in range(two):
                                    nc.sync.dma_start(bt[:, h],
                                                      _bsrc(h, kg, q))
                    else:
                        continue
                    if not do_mm:
                        continue
                    for i in range(4):
                        k = 4 * kg + i
                        # on the diagonal chunk only columns >= 128i are
                        # valid in SBUF (and B is zero left of them anyway)
                        c0 = 128 * i if kg == q else 0
                        for j in act:
                            if k < KSTART[j]:
                                continue
                            idx = AOFF[j] + (k - KSTART[j])
                            first = k == KSTART[j]
                            last = k == kend
                            if two == 1:
                                nc.tensor.matmul(
                                    psums[j][:, c0:], a_sb[:, 0, idx, :],
                                    bt[:, 0, i, c0:],
                                    start=first, stop=last)
                            else:
                                # hi@hi, hi@lo, lo@hi
                                for n3, (ha, hb) in enumerate(
                                        [(0, 0), (0, 1), (1, 0)]):
                                    nc.tensor.matmul(
                                        psums[j][:, c0:],
                                        a_sb[:, ha, idx, :],
                                        bt[:, hb, i, c0:],
                                        start=first and n3 == 0,
                                        stop=last and n3 == 2)
                for j in act:
                    if not (do_copy or do_store):
                        continue
                    t = PAIRS.index((j, q))
                    if variant == "vstore":
                        nc.sync.dma_start(
                            c_dram[t * P:(t + 1) * P, :], ot_fixed[:])
                        continue
                    ot = opool.tile([P, QW], mybir.dt.float32, tag="ot")
                    if do_mm:
                        nc.vector.tensor_copy(ot[:], psums[j][:])
                    else:
                        nc.vector.tensor_copy(
                            ot[:].rearrange("p (a b) -> p a b", a=4),
                            _asrc_f32(j))
                    if do_store:
                        # scalar (ACT) HWDGE ring: keeps compute-gated output
                        # stores out of the B-stream's SP FIFO
                        nc.scalar.dma_start(
                            c_dram[t * P:(t + 1) * P, :], ot[:])
    nc.compile()
    _nc_cache[(mode, rep, variant)] = nc
    return nc


def _split_bf16(x):
    import ml_dtypes
    hi = x.astype(ml_dtypes.bfloat16)
    lo = (x - hi.astype(np.float32)).astype(ml_dtypes.bfloat16)
    return hi, lo


def pack_inputs(A, B, mode=MODE):
    """Build per-core in_maps (partition-major packed layouts)."""
    A = np.ascontiguousarray(np.asarray(A, dtype=np.float32))
    B = np.ascontiguousarray(np.asarray(B, dtype=np.float32))
    two = 2 if mode == "bf16x3" else 1

    # B[128k+p, 512q+n] -> Bp[q, p, k, n] -> [NQ*P, NKT*QW]
    def _pack_b(x):
        return np.ascontiguousarray(
            x.reshape(NKT, P, NQ, QW).transpose(2, 1, 0, 3)
        ).reshape(NQ * P, NKT * QW)

    if mode == "bf16x3":
        hi, lo = _split_bf16(B)
        b_all = np.concatenate([_pack_b(hi), _pack_b(lo)], axis=0)
    else:
        b_all = _pack_b(B)

    in_maps = []
    for c in range(NCORES):
        ap = np.zeros((ATOT, P, P), np.float32)
        for j in range(NSLOT):
            b = 8 * j + c
            rb = P * b
            for k in range(max(KSTART[j], b), NKT):
                ap[AOFF[j] + k - KSTART[j]] = \
                    A[rb:rb + P, P * k:P * k + P].T
        # [t, p, m] -> [p, t, m] -> [P, ATOT*P]
        def _pack_a(x):
            return np.ascontiguousarray(
                x.transpose(1, 0, 2)).reshape(P, ATOT * P)

        if mode == "bf16x3":
            hi, lo = _split_bf16(ap)
            apk = np.concatenate([_pack_a(hi), _pack_a(lo)], axis=0)
        else:
            apk = _pack_a(ap)
        in_maps.append({"Apack": apk, "B": b_all})
    return in_maps


def unpack_output(results):
    C = np.zeros((N, N), np.float32)
    for c, r in enumerate(results):
        co = np.asarray(r["Cout"]).reshape(NT, P, QW)
        for t, (j, q) in enumerate(PAIRS):
            b = 8 * j + c
            C[P * b:P * b + P, QW * q:QW * q + QW] = co[t]
    return C


def kernel(A, B):
    nc = build_nc(MODE)
    in_maps = pack_inputs(A, B, MODE)
    res = bass_utils.run_bass_kernel_spmd(
        nc, in_maps, core_ids=list(range(NCORES)), trace=False)
    return unpack_output(res.results)



# revision 9
# speedup vs baseline: 1.4856x; 1.4856x over previous
"""Trainium2 Bass kernel: C = triu(A @ B), A/B upper-triangular 4096x4096 fp32.

Strategy (row-parallel over 8 cores, SPMD single program):
  * 32 row-blocks of 128 rows. Core c owns blocks {c, 8+c, 16+c, 24+c}
    ("slot" j = block 8j + c).
  * One uniform schedule for all cores: for column tile q (8 tiles of 512)
    and slot j, accumulate k-tiles k in [8j, 4q+3].  Per-core variation
    lives entirely in the DATA: the host packs A^T tiles per core and
    zero-fills tiles with k < own-block, so padded matmuls contribute
    exact zeros.  Since A and B are both upper-triangular, the lower
    triangle of C comes out exactly 0 - no masking needed.
  * A^T pack (80 tiles of 128x128) is cached in SBUF; B streams once per
    column tile with below-diagonal tiles skipped.
"""

import numpy as np
from contextlib import ExitStack

import concourse.mybir as mybir
import concourse.tile as tile
from concourse import bacc, bass_utils

N = 4096
P = 128
NCORES = 8
NSLOT = 4          # row-block slots per core
NQ = 8             # 512-wide output column tiles
QW = 512
NKT = 32           # 128-wide k tiles
KSTART = [0, 8, 16, 24]            # first k-tile per slot (min over cores)
ANT = [32, 24, 16, 8]              # k-tiles stored per slot
AOFF = [0, 32, 56, 72]             # slot offsets into the A pack
ATOT = 80                          # total packed A tiles per core

# (slot, qtile) pairs the program computes/writes, in emission order
PAIRS = [(j, q) for q in range(NQ) for j in range(NSLOT) if 4 * q + 4 > 8 * j]
NT = len(PAIRS)                    # 20 output tiles of 128x512 per core

# matmul dtype mode: "fp16" (half DMA bytes, ~5e-4 rel err), "fp32r"
# (fast, ~11-bit mantissa), "bf16x3" (hi/lo 3-pass split, near-fp32
# accuracy), "fp32" (exact, 4x slower PE)
MODE = "fp16"
# store C as fp16 (halves output DMA; adds ~2^-11 rel err)
OUT16 = True

# pool buffer counts (double/triple buffering)
BUFS_B = 3
BUFS_O = 4
BUFS_PS = 8

_nc_cache = {}


def build_nc(mode=MODE, rep=1, variant="full"):
    """rep>1 repeats the whole compute (for dispatch-overhead-cancelling
    timing): T_hw ~= (T(rep=R) - T(rep=1)) / (R-1).
    variant: "full" | "nomm" (DMAs only) | "nodma" (matmuls only)."""
    if (mode, rep, variant) in _nc_cache:
        return _nc_cache[(mode, rep, variant)]
    two = 2 if mode == "bf16x3" else 1
    dt_in = {
        "fp16": mybir.dt.float16,
        "fp32r": mybir.dt.float32r,
        "bf16x3": mybir.dt.bfloat16,
        "fp32": mybir.dt.float32,
    }[mode]
    dt_out = mybir.dt.float16 if OUT16 else mybir.dt.float32

    nc = bacc.Bacc("TRN2", target_bir_lowering=False, debug=False,
                   num_devices=NCORES)
    # partition-major packed layouts (see pack_inputs): per-partition data is
    # contiguous so every DMA is 128 descriptors of large contiguous runs.
    # Apack row = h*P + p(k-within-tile), col = t*P + m  (40KB/partition)
    a_dram = nc.dram_tensor("Apack", [two * P, ATOT * P], dt_in,
                            kind="ExternalInput").ap()
    # B row = (h*NQ + q)*P + p, col = k*QW + n          (8KB runs/partition)
    b_dram = nc.dram_tensor("B", [two * NQ * P, NKT * QW], dt_in,
                            kind="ExternalInput").ap()
    c_dram = nc.dram_tensor("Cout", [NT * P, QW], dt_out,
                            kind="ExternalOutput").ap()

    with tile.TileContext(nc) as tc:
        with ExitStack() as ctx:
            apool = ctx.enter_context(tc.tile_pool(name="apool", bufs=1))
            bpool = ctx.enter_context(tc.tile_pool(name="bpool", bufs=BUFS_B))
            opool = ctx.enter_context(tc.tile_pool(name="opool", bufs=BUFS_O))
            pspool = ctx.enter_context(
                tc.tile_pool(name="pspool", bufs=BUFS_PS, space="PSUM"))

            do_bdma = variant in ("full", "nomm", "vbdma")
            do_mm = variant in ("full", "nodma", "vmm")
            do_copy = variant in ("full", "nomm", "nodma", "vcopy")
            do_store = variant in ("full", "nomm", "nodma", "vstore")

            # A load split so early matmuls are gated only by the tiles they
            # read: slot0 k0..7 (feeds q=1/q=0) lands in ~1.5us, the rest
            # overlaps with the B stream.
            a_sb = apool.tile([P, two, ATOT, P], dt_in)
            for t0, t1 in [(0, 8), (8, 32), (32, ATOT)]:
                for h in range(two):
                    nc.sync.dma_start(
                        a_sb[:, h, t0:t1, :],
                        a_dram[h * P:(h + 1) * P, t0 * P:t1 * P].rearrange(
                            "p (t m) -> p t m", m=P))

            # micro variants: per rep emit n tiny ops, skip the main loop
            micro = variant.startswith("vd") or variant in ("vgps8", "vdve8")
            if micro:
                n_ops = (8 if variant in ("vgps8", "vdve8")
                         else int(variant[2:]))
                mpool = ctx.enter_context(tc.tile_pool(name="mp", bufs=16))
                for r in range(rep):
                    for i in range(n_ops):
                        mt = mpool.tile([P, QW], mybir.dt.float32, tag="mt",
                                        name=f"mt_{r}_{i}")
                        if variant == "vdve8":
                            src = a_sb[:, 0, 4 * i:4 * i + 4, :]
                            if dt_in == mybir.dt.float32r:
                                src = src.bitcast(mybir.dt.float32)
                            nc.vector.tensor_copy(
                                mt[:].rearrange("p (a b) -> p a b", a=4),
                                src)
                        elif variant == "vgps8":
                            nc.gpsimd.dma_start(
                                mt[:],
                                b_dram[i * P:(i + 1) * P, 0:QW]
                                .bitcast(mybir.dt.float32))
                        else:
                            nc.sync.dma_start(
                                mt[:],
                                b_dram[i * P:(i + 1) * P, 0:QW]
                                .bitcast(mybir.dt.float32))
            bt_fixed = None
            ot_fixed = None

            def _asrc_f32(j):
                src = a_sb[:, 0, 4 * j:4 * j + 4, :]
                if dt_in == mybir.dt.float32r:
                    src = src.bitcast(mybir.dt.float32)
                return src

            if variant == "vstore":
                ot_fixed = opool.tile([P, QW], dt_out,
                                      name="ot_fixed")
                nc.vector.tensor_copy(
                    ot_fixed[:].rearrange("p (a b) -> p a b", a=4),
                    _asrc_f32(0))

            def _bsrc(h, kg, q):
                return b_dram[
                    (h * NQ + q) * P:(h * NQ + q + 1) * P,
                    4 * kg * QW:(4 * kg + 4) * QW,
                ].rearrange("p (ko n) -> p ko n", ko=4)

            def _load_diag_chunk(bt, q):
                # per k-row load only the valid columns [128i, 512) -
                # below-diagonal 128-blocks of B are zero
                for h in range(two):
                    for i in range(4):
                        row = (h * NQ + q) * P
                        col = (4 * q + i) * QW + 128 * i
                        nc.sync.dma_start(
                            bt[:, h, i, 128 * i:],
                            b_dram[row:row + P, col:col + QW - 128 * i])

            # q=0's only chunk (0.6MB) is consumed last (Q_ORDER ends on 0):
            # prefetch it into a dedicated buffer at the start so the tail
            # never waits on DMA
            # (tried: prefetching q=0's chunk at the head — model-worse by
            # 1.8us, the DMA stream is saturated so early bytes displace
            # the critical sequence)
            bt_q0 = None

            # q order: q=1 first (ready after the small A-head load), then
            # heaviest-to-lightest so the schedule drains into the tiny q=0
            # tail (4 matmuls + 1 copy + 1 store). Model-swept optimum.
            Q_ORDER = globals().get("_Q_ORDER_OVERRIDE") or \
                [1, 7, 6, 5, 4, 3, 2, 0]
            for _r, q in ([] if micro else
                          [(r, q) for r in range(rep) for q in Q_ORDER]):
                act = [j for j in range(NSLOT) if 4 * q + 4 > 8 * j]
                psums = {
                    j: pspool.tile([P, QW], mybir.dt.float32, tag="ps",
                                   name=f"ps_{_r}_{q}_{j}")
                    for j in act
                } if do_mm else {}
                kend = 4 * q + 3
                for kg in range(q + 1):
                    if do_mm and not do_bdma:
                        if bt_fixed is None:
                            bt_fixed = bpool.tile([P, two, 4, QW], dt_in,
                                                  tag="bt", name="bt_fixed")
                            for h in range(two):
                                nc.sync.dma_start(bt_fixed[:, h],
                                                  _bsrc(h, 0, 0))
                        bt = bt_fixed
                    elif do_bdma or variant == "vmin":
                        if variant == "vmin" and kg > 0:
                            continue
                        if bt_q0 is not None and q == 0:
                            bt = bt_q0
                        else:
                            bt = bpool.tile([P, two, 4, QW], dt_in,
                                            tag="bt")
                            if kg == q:
                                _load_diag_chunk(bt, q)
                            else:
                                for h in range(two):
                                    nc.sync.dma_start(bt[:, h],
                                                      _bsrc(h, kg, q))
                    else:
                        continue
                    if not do_mm:
                        continue
                    for i in range(4):
                        k = 4 * kg + i
                        # on the diagonal chunk only columns >= 128i are
                        # valid in SBUF (and B is zero left of them anyway)
                        c0 = 128 * i if kg == q else 0
                        for j in act:
                            if k < KSTART[j]:
                                continue
                            idx = AOFF[j] + (k - KSTART[j])
                            first = k == KSTART[j]
                            last = k == kend
                            if two == 1:
                                nc.tensor.matmul(
                                    psums[j][:, c0:], a_sb[:, 0, idx, :],
                                    bt[:, 0, i, c0:],
                                    start=first, stop=last)
                            else:
                                # hi@hi, hi@lo, lo@hi
                                for n3, (ha, hb) in enumerate(
                                        [(0, 0), (0, 1), (1, 0)]):
                                    nc.tensor.matmul(
                                        psums[j][:, c0:],
                                        a_sb[:, ha, idx, :],
                                        bt[:, hb, i, c0:],
                                        start=first and n3 == 0,
                                        stop=last and n3 == 2)
                for j in act:
                    if not (do_copy or do_store):
                        continue
                    t = PAIRS.index((j, q))
                    if variant == "vstore":
                        nc.sync.dma_start(
                            c_dram[t * P:(t + 1) * P, :], ot_fixed[:])
                        continue
                    ot = opool.tile([P, QW], dt_out, tag="ot")
                    if do_mm:
                        nc.vector.tensor_copy(ot[:], psums[j][:])
                    else:
                        nc.vector.tensor_copy(
                            ot[:].rearrange("p (a b) -> p a b", a=4),
                            _asrc_f32(j))
                    if do_store:
                        # scalar (ACT) HWDGE ring: keeps compute-gated output
                        # stores out of the B-stream's SP FIFO
                        nc.scalar.dma_start(
                            c_dram[t * P:(t + 1) * P, :], ot[:])
    nc.compile()
    _nc_cache[(mode, rep, variant)] = nc
    return nc


def _split_bf16(x):
    import ml_dtypes
    hi = x.astype(ml_dtypes.bfloat16)
    lo = (x - hi.astype(np.float32)).astype(ml_dtypes.bfloat16)
    return hi, lo


def pack_inputs(A, B, mode=MODE):
    """Build per-core in_maps (partition-major packed layouts)."""
    A = np.ascontiguousarray(np.asarray(A, dtype=np.float32))
    B = np.ascontiguousarray(np.asarray(B, dtype=np.float32))
    two = 2 if mode == "bf16x3" else 1

    # B[128k+p, 512q+n] -> Bp[q, p, k, n] -> [NQ*P, NKT*QW]
    def _pack_b(x):
        return np.ascontiguousarray(
            x.reshape(NKT, P, NQ, QW).transpose(2, 1, 0, 3)
        ).reshape(NQ * P, NKT * QW)

    if mode == "bf16x3":
        hi, lo = _split_bf16(B)
        b_all = np.concatenate([_pack_b(hi), _pack_b(lo)], axis=0)
    elif mode == "fp16":
        b_all = _pack_b(B).astype(np.float16)
    else:
        b_all = _pack_b(B)

    in_maps = []
    for c in range(NCORES):
        ap = np.zeros((ATOT, P, P), np.float32)
        for j in range(NSLOT):
            b = 8 * j + c
            rb = P * b
            for k in range(max(KSTART[j], b), NKT):
                ap[AOFF[j] + k - KSTART[j]] = \
                    A[rb:rb + P, P * k:P * k + P].T
        # [t, p, m] -> [p, t, m] -> [P, ATOT*P]
        def _pack_a(x):
            return np.ascontiguousarray(
                x.transpose(1, 0, 2)).reshape(P, ATOT * P)

        if mode == "bf16x3":
            hi, lo = _split_bf16(ap)
            apk = np.concatenate([_pack_a(hi), _pack_a(lo)], axis=0)
        elif mode == "fp16":
            apk = _pack_a(ap).astype(np.float16)
        else:
            apk = _pack_a(ap)
        in_maps.append({"Apack": apk, "B": b_all})
    return in_maps


def unpack_output(results):
    C = np.zeros((N, N), np.float32)
    for c, r in enumerate(results):
        co = np.asarray(r["Cout"]).astype(np.float32).reshape(NT, P, QW)
        for t, (j, q) in enumerate(PAIRS):
            b = 8 * j + c
            C[P * b:P * b + P, QW * q:QW * q + QW] = co[t]
    return C


def kernel(A, B):
    nc = build_nc(MODE)
    in_maps = pack_inputs(A, B, MODE)
    res = bass_utils.run_bass_kernel_spmd(
        nc, in_maps, core_ids=list(range(NCORES)), trace=False)
    return unpack_output(res.results)



# revision 15
# speedup vs baseline: 1.6499x; 1.1106x over previous
"""Trainium2 Bass kernel: C = triu(A @ B), A/B upper-triangular 4096x4096 fp32.

Strategy (4 row-groups x 2 column-groups over 8 cores, SPMD single program):
  * 32 row-blocks of 128 rows. Core (g, s) owns blocks {4j + g : j=0..7}
    (slot j) and column tiles q = 2t + s (t=0..3, 512 cols each).
  * One uniform schedule: for column slot t and row slot j < 2t+2,
    accumulate k-tiles k in [4j, 8t+7]. Per-core variation lives in the
    DATA: the host packs A^T tiles per core (zero-filled below the
    diagonal) and B columns for its own q's; k-tiles past a core's
    diagonal are structural zeros of triu(B), so padded matmuls
    contribute exact zeros. Scheduled matmuls: 280 (the SPMD floor).
  * The 2-way column split halves the B stream (10 chunk-columns vs 36):
    per-core DMA = A 4.7MB + B ~9.7MB + C 2.6MB = 48us < PE ~57us, so
    the kernel is PE-bound (vs B-replicated row-parallel at 64us DMA).
  * fp16 inputs + fp16 output: the cost model runs fp16 matmuls at the
    same 1 cycle/row as fp32r, so halving every DMA byte is free. PSUM
    accumulation stays fp32; measured rel err ~5e-4 (budget 2e-2).
  * DMA instruction count kept low (each costs a ~627ns hold of the
    shared HWDGE issue port): B streams in 2-chunk (1MiB) DMAs, the
    last chunk of each t is 2 rectangles (its left-of-diagonal halves
    are zero for both column groups), output tiles stored in pairs.
  * A pack is laid out k-major (all slots' tiles for k-group kg
    together) so the A stream lands in lockstep with B consumption.
"""

import numpy as np
from contextlib import ExitStack

import concourse.mybir as mybir
import concourse.tile as tile
from concourse import bacc, bass_utils

N = 4096
P = 128
NCORES = 8
NROWG = 4          # row groups (cores per column group)
NCOLG = 2          # column groups
NSLOT = 8          # row-block slots per core (blocks 4j + g)
NT_COL = 4         # column tiles per core (q = 2t + s)
QW = 512
NKT = 32           # 128-wide k tiles

# A pack: tile (j, k), k in [4j, 31], laid out k-group-major:
# idx = 2*kg*(kg+1) + 4*j + (k - 4*kg), kg = k // 4  (slots j <= kg)
ATOT = 144


def _aidx(j, k):
    kg = k // 4
    return 2 * kg * (kg + 1) + 4 * j + (k - 4 * kg)


# t order: ramps B demand 1.5/2/3/4 MB while PE work ramps 2.5/8.5/18/31us
T_ORDER = [0, 1, 2, 3]

# (row slot, col slot) pairs in EMISSION order; output tile t = index here
EMIT_PAIRS = [(j, t) for t in T_ORDER for j in range(NSLOT)
              if j < 2 * t + 2]
NT = len(EMIT_PAIRS)               # 20 output tiles of 128x512 per core

# matmul dtype mode: "fp16" (half DMA bytes, ~5e-4 rel err), "fp32r"
# (fast, ~11-bit mantissa), "fp32" (exact, 4x slower PE)
MODE = "fp16"
# store C as fp16 (halves output DMA; adds ~2^-11 rel err)
OUT16 = True

# pool buffer counts (double/triple buffering)
BUFS_B = 6
BUFS_O = 6
BUFS_PS = 8

_nc_cache = {}


def _groups(t):
    """B k-chunk DMA groups for column slot t: 2-chunk pairs, then a
    leftover single, then the final chunk (kg == 2t+1) last (it gets
    the 2-rect diagonal load + matmul column narrowing)."""
    last = 2 * t + 1
    gs = []
    kg = 0
    while kg < last:
        nk = min(2, last - kg)
        gs.append((kg, nk, False))
        kg += nk
    gs.append((last, 1, True))
    return gs


def build_nc(mode=MODE, rep=1, variant="full"):
    """variant: "full" | "nomm" (DMAs only) | "nodma" (matmuls only)."""
    if (mode, rep, variant) in _nc_cache:
        return _nc_cache[(mode, rep, variant)]
    dt_in = {
        "fp16": mybir.dt.float16,
        "fp32r": mybir.dt.float32r,
        "fp32": mybir.dt.float32,
    }[mode]
    dt_out = mybir.dt.float16 if OUT16 else mybir.dt.float32

    nc = bacc.Bacc("TRN2", target_bir_lowering=False, debug=False,
                   num_devices=NCORES)
    # partition-major packed layouts (see pack_inputs): per-partition data is
    # contiguous so every DMA is 128 descriptors of large contiguous runs.
    # Apack row = p, col = idx*P + m (idx per _aidx)
    a_dram = nc.dram_tensor("Apack", [P, ATOT * P], dt_in,
                            kind="ExternalInput").ap()
    # B row = t*P + p, col = k*QW + n
    b_dram = nc.dram_tensor("B", [NT_COL * P, NKT * QW], dt_in,
                            kind="ExternalInput").ap()
    c_dram = nc.dram_tensor("Cout", [NT * P, QW], dt_out,
                            kind="ExternalOutput").ap()

    with tile.TileContext(nc) as tc:
        with ExitStack() as ctx:
            apool = ctx.enter_context(tc.tile_pool(name="apool", bufs=1))
            bpool = ctx.enter_context(tc.tile_pool(name="bpool", bufs=BUFS_B))
            opool = ctx.enter_context(tc.tile_pool(name="opool", bufs=BUFS_O))
            pspool = ctx.enter_context(
                tc.tile_pool(name="pspool", bufs=BUFS_PS, space="PSUM"))

            do_bdma = variant in ("full", "nomm")
            do_mm = variant in ("full", "nodma")
            do_copy = variant in ("full", "nomm", "nodma")
            do_store = variant in ("full", "nomm", "nodma")

            # A stream in k-group ranges, aligned with B-consumption order:
            # t consumes A k-groups kg <= 2t+1.
            a_sb = apool.tile([P, ATOT, P], dt_in)
            for g0, g1 in [(0, 2), (2, 4), (4, 6), (6, 8)]:
                i0, i1 = 2 * g0 * (g0 + 1), 2 * g1 * (g1 + 1)
                nc.sync.dma_start(
                    a_sb[:, i0:i1, :],
                    a_dram[:, i0 * P:i1 * P].rearrange(
                        "p (t m) -> p t m", m=P))

            bt_fixed = None
            ot_cur = [None]   # pair-store buffer, carried across col slots

            for _r, t in [(r, t) for r in range(rep) for t in T_ORDER]:
                act = [j for j in range(NSLOT) if j < 2 * t + 2]
                psums = {
                    j: pspool.tile([P, QW], mybir.dt.float32, tag="ps",
                                   name=f"ps_{_r}_{t}_{j}")
                    for j in act
                } if do_mm else {}
                kend = 8 * t + 7
                for kg0, nk, islast in _groups(t):
                    if do_mm and not do_bdma:
                        if bt_fixed is None:
                            bt_fixed = bpool.tile([P, 2, 4, QW], dt_in,
                                                  tag="bt", name="bt_fixed")
                            nc.sync.dma_start(
                                bt_fixed[:],
                                b_dram[0:P, 0:8 * QW].rearrange(
                                    "p (kg ko n) -> p kg ko n", kg=2, ko=4))
                        bt = bt_fixed
                        rd = lambda kg, i, c0: bt_fixed[:, 0, i, c0:]
                    elif do_bdma:
                        bt = bpool.tile([P, 2, 4, QW], dt_in, tag="bt")
                        row = t * P
                        if islast:
                            # 2 rects; left-of-diagonal halves are zero in
                            # dram (triu) so over-reading columns is exact
                            base = 4 * kg0 * QW
                            nc.sync.dma_start(
                                bt[:, 0, 0:2, :],
                                b_dram[row:row + P, base:base + 2 * QW]
                                .rearrange("p (ko n) -> p ko n", ko=2))
                            nc.sync.dma_start(
                                bt[:, 0, 2:4, 256:],
                                b_dram[row:row + P,
                                       base + 2 * QW:base + 4 * QW]
                                .rearrange("p (ko n) -> p ko n",
                                           ko=2)[:, :, 256:])
                        else:
                            nc.sync.dma_start(
                                bt[:, 0:nk],
                                b_dram[row:row + P,
                                       4 * kg0 * QW:(4 * kg0 + 4 * nk) * QW]
                                .rearrange("p (kg ko n) -> p kg ko n",
                                           kg=nk, ko=4))
                        rd = lambda kg, i, c0, bt=bt, kg0=kg0: \
                            bt[:, kg - kg0, i, c0:]
                    else:
                        continue
                    if not do_mm:
                        continue
                    # final chunk j-major: each psum[j] finishes (stop flag)
                    # after its own 4 matmuls, so copies/stores drain while
                    # later slots still accumulate
                    if islast:
                        order = [(kg0, i, j) for j in act for i in range(4)]
                    else:
                        order = [(kg, i, j)
                                 for kg in range(kg0, kg0 + nk)
                                 for i in range(4) for j in act]
                    for kg, i, j in order:
                        k = 4 * kg + i
                        if k < 4 * j:
                            continue
                        # on the final chunk only columns >= 128i carry
                        # data (B is zero left of them for both col groups)
                        c0 = 128 * i if islast else 0
                        nc.tensor.matmul(
                            psums[j][:, c0:], a_sb[:, _aidx(j, k), :],
                            rd(kg, i, c0),
                            start=(k == 4 * j), stop=(k == kend))
                for j in act:
                    if not (do_copy or do_store):
                        continue
                    te = EMIT_PAIRS.index((j, t))
                    if te % 2 == 0:
                        ot_cur[0] = opool.tile([P, 2, QW], dt_out,
                                               tag="ot",
                                               name=f"ot_{_r}_{te}")
                    ot = ot_cur[0]
                    if do_mm:
                        nc.vector.tensor_copy(ot[:, te % 2], psums[j][:])
                    else:
                        nc.vector.tensor_copy(
                            ot[:, te % 2].rearrange("p (a b) -> p a b", a=4),
                            a_sb[:, 0:4, :])
                    if do_store and te % 2 == 1:
                        t0 = te - 1
                        # scalar (ACT) HWDGE ring: keeps compute-gated output
                        # stores out of the B-stream's SP FIFO
                        nc.scalar.dma_start(
                            c_dram[t0 * P:(t0 + 2) * P, :].rearrange(
                                "(t p) n -> p t n", t=2),
                            ot[:])
    nc.compile()
    _nc_cache[(mode, rep, variant)] = nc
    return nc


def _core_gs(c):
    """Core id -> (row group g, column group s)."""
    return c // NCOLG, c % NCOLG


def pack_inputs(A, B, mode=MODE):
    """Build per-core in_maps (partition-major packed layouts)."""
    A = np.ascontiguousarray(np.asarray(A, dtype=np.float32))
    B = np.ascontiguousarray(np.asarray(B, dtype=np.float32))
    np_in = {"fp16": np.float16, "fp32r": np.float32,
             "fp32": np.float32}[mode]

    # B[128k+p, 512q+n] -> per column group s: rows t*P+p, cols k*QW+n
    B4 = B.reshape(NKT, P, 8, QW)     # [k, p, q, n]
    b_packs = []
    for s in range(NCOLG):
        qs = [2 * t + s for t in range(NT_COL)]
        bp = np.ascontiguousarray(
            B4[:, :, qs, :].transpose(2, 1, 0, 3)   # [t, p, k, n]
        ).reshape(NT_COL * P, NKT * QW).astype(np_in)
        b_packs.append(bp)

    in_maps = []
    for c in range(NCORES):
        g, s = _core_gs(c)
        ap = np.zeros((ATOT, P, P), np.float32)
        for j in range(NSLOT):
            b = 4 * j + g
            rb = P * b
            for k in range(max(4 * j, b), NKT):
                ap[_aidx(j, k)] = A[rb:rb + P, P * k:P * k + P].T
        apk = np.ascontiguousarray(
            ap.transpose(1, 0, 2)).reshape(P, ATOT * P).astype(np_in)
        in_maps.append({"Apack": apk, "B": b_packs[s]})
    return in_maps


def unpack_output(results):
    C = np.zeros((N, N), np.float32)
    for c, r in enumerate(results):
        g, s = _core_gs(c)
        co = np.asarray(r["Cout"]).astype(np.float32).reshape(NT, P, QW)
        for te, (j, t) in enumerate(EMIT_PAIRS):
            b = 4 * j + g
            q = 2 * t + s
            C[P * b:P * b + P, QW * q:QW * q + QW] = co[te]
    return C


def kernel(A, B):
    nc = build_nc(MODE)
    in_maps = pack_inputs(A, B, MODE)
    res = bass_utils.run_bass_kernel_spmd(
        nc, in_maps, core_ids=list(range(NCORES)), trace=False)
    return unpack_output(res.results)


# revision 30
# speedup vs baseline: 1.9667x; 1.1920x over previous
"""Trainium2 Bass kernel: C = triu(A @ B), A/B upper-triangular 4096x4096 fp32.

Strategy (4 row-groups x 2 column-groups over 8 cores, SPMD single program):
  * 32 row-blocks of 128 rows. Core (g, s) owns blocks {4j + g : j=0..7}
    (slot j) and column tiles q = 2t + s (t=0..3, 512 cols each).
  * One uniform schedule: for column slot t and row slot j < 2t+2,
    accumulate k-tiles k in [4j, 8t+7]. Per-core variation lives in the
    DATA: the host packs A^T tiles per core (zero-filled below the
    diagonal) and B columns for its own q's; k-tiles past a core's
    diagonal are structural zeros of triu(B), so padded matmuls
    contribute exact zeros. Scheduled matmuls: 280 (the SPMD floor).
  * The 2-way column split halves the B stream (10 chunk-columns vs 36):
    per-core DMA = A 4.7MB + B ~9.7MB + C 2.6MB = 48us < PE ~57us, so
    the kernel is PE-bound (vs B-replicated row-parallel at 64us DMA).
  * fp16 inputs + fp16 output: the cost model runs fp16 matmuls at the
    same 1 cycle/row as fp32r, so halving every DMA byte is free. PSUM
    accumulation stays fp32; measured rel err ~5e-4 (budget 2e-2).
  * DMA instruction count kept low (each costs a ~627ns hold of the
    shared HWDGE issue port): B streams in 2-chunk (1MiB) DMAs, the
    last chunk of each t is 2 rectangles (its left-of-diagonal halves
    are zero for both column groups), output tiles stored in pairs.
  * A pack is laid out k-major (all slots' tiles for k-group kg
    together) so the A stream lands in lockstep with B consumption.
"""

import numpy as np
from contextlib import ExitStack

import concourse.mybir as mybir
import concourse.tile as tile
from concourse import bacc, bass_utils

N = 4096
P = 128
NCORES = 8
NROWG = 4          # row groups (cores per column group)
NCOLG = 2          # column groups
NSLOT = 8          # row-block slots per core (blocks 4j + g)
NT_COL = 4         # column tiles per core (q = 2t + s)
QW = 512
NKT = 32           # 128-wide k tiles

# A pack: tile (j, k), k in [4j, 31], laid out k-group-major:
# idx = 2*kg*(kg+1) + 4*j + (k - 4*kg), kg = k // 4  (slots j <= kg)
ATOT = 144


def _aidx(j, k):
    kg = k // 4
    return 2 * kg * (kg + 1) + 4 * j + (k - 4 * kg)


# t order: t=1 first (its 8.5us of PE work covers t=0's loads; t=0 has
# less compute than DMA so it must not lead), then ramp up
T_ORDER = [1, 0, 2, 3]

# (row slot, col slot) pairs in EMISSION order (descending j: psum[j]
# completes right after chunk kg=j); output tile index = index here
EMIT_PAIRS = [(j, t) for t in T_ORDER for j in reversed(range(2 * t + 2))]
NT = len(EMIT_PAIRS)               # 20 output tiles of 128x512 per core

# matmul dtype mode: "fp16" (half DMA bytes, ~5e-4 rel err), "fp32r"
# (fast, ~11-bit mantissa), "fp32" (exact, 4x slower PE)
MODE = "fp16"
# store C as fp16 (halves output DMA; adds ~2^-11 rel err)
OUT16 = True

# pool buffer counts (double/triple buffering)
BUFS_B = 6
BUFS_O = 6
BUFS_PS = 8

_nc_cache = {}


def build_nc(mode=MODE, rep=1, variant="full"):
    """variant: "full" | "nomm" (DMAs only) | "nodma" (matmuls only)."""
    if (mode, rep, variant) in _nc_cache:
        return _nc_cache[(mode, rep, variant)]
    dt_in = {
        "fp16": mybir.dt.float16,
        "fp32r": mybir.dt.float32r,
        "fp32": mybir.dt.float32,
    }[mode]
    dt_out = mybir.dt.float16 if OUT16 else mybir.dt.float32

    nc = bacc.Bacc("TRN2", target_bir_lowering=False, debug=False,
                   num_devices=NCORES)
    # partition-major packed layouts (see pack_inputs): per-partition data is
    # contiguous so every DMA is 128 descriptors of large contiguous runs.
    # Apack row = p, col = idx*P + m (idx per _aidx)
    a_dram = nc.dram_tensor("Apack", [P, ATOT * P], dt_in,
                            kind="ExternalInput").ap()
    # B row = t*P + p, col = k*QW + n
    b_dram = nc.dram_tensor("B", [NT_COL * P, NKT * QW], dt_in,
                            kind="ExternalInput").ap()
    c_dram = nc.dram_tensor("Cout", [NT * P, QW], dt_out,
                            kind="ExternalOutput").ap()

    with tile.TileContext(nc) as tc:
        with ExitStack() as ctx:
            apool = ctx.enter_context(tc.tile_pool(name="apool", bufs=1))
            bpool = ctx.enter_context(tc.tile_pool(name="bpool", bufs=BUFS_B))
            opool = ctx.enter_context(tc.tile_pool(name="opool", bufs=BUFS_O))
            pspool = ctx.enter_context(
                tc.tile_pool(name="pspool", bufs=BUFS_PS, space="PSUM"))

            do_bdma = variant in ("full", "nomm")
            do_mm = variant in ("full", "nodma")
            do_copy = variant in ("full", "nomm", "nodma")
            do_store = variant in ("full", "nomm", "nodma")

            # A streams per k-group, on demand right before the first B
            # chunk whose matmuls read it (chunk kg needs A k-group kg).
            a_sb = apool.tile([P, ATOT, P], dt_in)
            a_have = set()

            def _need_a(kg):
                if kg in a_have:
                    return
                a_have.add(kg)
                i0, i1 = 2 * kg * (kg + 1), 2 * (kg + 1) * (kg + 2)
                nc.sync.dma_start(
                    a_sb[:, i0:i1, :],
                    a_dram[:, i0 * P:i1 * P].rearrange(
                        "p (t m) -> p t m", m=P))

            bt_fixed = None
            ot_cur = [None]   # pair-store buffer, carried across col slots

            def _emit_out(_r, t, j, psums):
                """psum[j] complete: copy to SBUF (fp16) and pair-store."""
                te = EMIT_PAIRS.index((j, t))
                if te % 2 == 0:
                    ot_cur[0] = opool.tile([P, 2, QW], dt_out, tag="ot",
                                           name=f"ot_{_r}_{te}")
                ot = ot_cur[0]
                if do_mm:
                    # alternate copy engines (DVE / ACT) so the copy
                    # stream never serializes behind one engine
                    if te % 2 == 0:
                        nc.vector.tensor_copy(ot[:, 0], psums[j][:])
                    else:
                        nc.scalar.copy(ot[:, 1], psums[j][:])
                else:
                    nc.vector.tensor_copy(
                        ot[:, te % 2].rearrange("p (a b) -> p a b", a=4),
                        a_sb[:, 0:4, :])
                if do_store and te % 2 == 1:
                    t0 = te - 1
                    # scalar (ACT) HWDGE ring: keeps compute-gated output
                    # stores out of the B-stream's SP FIFO
                    nc.scalar.dma_start(
                        c_dram[t0 * P:(t0 + 2) * P, :].rearrange(
                            "(t p) n -> p t n", t=2),
                        ot[:])

            for _r, t in [(r, t) for r in range(rep) for t in T_ORDER]:
                act = [j for j in range(NSLOT) if j < 2 * t + 2]
                psums = {
                    j: pspool.tile([P, QW], mybir.dt.float32, tag="ps",
                                   name=f"ps_{_r}_{t}_{j}")
                    for j in act
                } if do_mm else {}
                last = 2 * t + 1
                # Chunks DESCENDING: the diagonal rect chunk (kg=last,
                # fewest bytes) feeds ALL 2t+2 row slots, so each t opens
                # with its fattest compute-per-byte chunk; psum[j] finishes
                # right after chunk kg=j, spreading copies/stores evenly.
                for kg in range(last, -1, -1):
                    if do_mm and not do_bdma:
                        if bt_fixed is None:
                            for g in range(NSLOT):
                                _need_a(g)
                            bt_fixed = bpool.tile([P, 4, QW], dt_in,
                                                  tag="bt", name="bt_fixed")
                            nc.sync.dma_start(
                                bt_fixed[:],
                                b_dram[0:P, 0:4 * QW].rearrange(
                                    "p (ko n) -> p ko n", ko=4))
                        bt = bt_fixed
                    elif do_bdma:
                        _need_a(kg)
                        bt = bpool.tile([P, 4, QW], dt_in, tag="bt")
                        row = t * P
                        base = 4 * kg * QW
                        if kg == last:
                            # 2 rects; left-of-diagonal halves are zero in
                            # dram (triu) so over-reading columns is exact
                            nc.sync.dma_start(
                                bt[:, 0:2, :],
                                b_dram[row:row + P, base:base + 2 * QW]
                                .rearrange("p (ko n) -> p ko n", ko=2))
                            nc.sync.dma_start(
                                bt[:, 2:4, 256:],
                                b_dram[row:row + P,
                                       base + 2 * QW:base + 4 * QW]
                                .rearrange("p (ko n) -> p ko n",
                                           ko=2)[:, :, 256:])
                        else:
                            nc.sync.dma_start(
                                bt[:],
                                b_dram[row:row + P, base:base + 4 * QW]
                                .rearrange("p (ko n) -> p ko n", ko=4))
                    else:
                        continue
                    if not do_mm:
                        continue
                    for i in range(4):
                        k = 4 * kg + i
                        # on the rect chunk only columns >= 128i carry
                        # data (B is zero left of them for both col groups)
                        c0 = 128 * i if kg == last else 0
                        for j in range(min(kg, last) + 1):
                            nc.tensor.matmul(
                                psums[j][:, c0:], a_sb[:, _aidx(j, k), :],
                                bt[:, i, c0:],
                                start=(kg == last and i == 0),
                                stop=(kg == j and i == 3))
                    if do_copy or do_store:
                        if kg <= last:
                            _emit_out(_r, t, kg, psums)
                if not do_mm and (do_copy or do_store):
                    for j in reversed(act):
                        _emit_out(_r, t, j, psums)
    nc.compile()
    _nc_cache[(mode, rep, variant)] = nc
    return nc


def _core_gs(c):
    """Core id -> (row group g, column group s)."""
    return c // NCOLG, c % NCOLG


def pack_inputs(A, B, mode=MODE):
    """Build per-core in_maps (partition-major packed layouts)."""
    A = np.ascontiguousarray(np.asarray(A, dtype=np.float32))
    B = np.ascontiguousarray(np.asarray(B, dtype=np.float32))
    np_in = {"fp16": np.float16, "fp32r": np.float32,
             "fp32": np.float32}[mode]

    # B[128k+p, 512q+n] -> per column group s: rows t*P+p, cols k*QW+n
    B4 = B.reshape(NKT, P, 8, QW)     # [k, p, q, n]
    b_packs = []
    for s in range(NCOLG):
        qs = [2 * t + s for t in range(NT_COL)]
        bp = np.ascontiguousarray(
            B4[:, :, qs, :].transpose(2, 1, 0, 3)   # [t, p, k, n]
        ).reshape(NT_COL * P, NKT * QW).astype(np_in)
        b_packs.append(bp)

    in_maps = []
    for c in range(NCORES):
        g, s = _core_gs(c)
        ap = np.zeros((ATOT, P, P), np.float32)
        for j in range(NSLOT):
            b = 4 * j + g
            rb = P * b
            for k in range(max(4 * j, b), NKT):
                ap[_aidx(j, k)] = A[rb:rb + P, P * k:P * k + P].T
        apk = np.ascontiguousarray(
            ap.transpose(1, 0, 2)).reshape(P, ATOT * P).astype(np_in)
        in_maps.append({"Apack": apk, "B": b_packs[s]})
    return in_maps


def unpack_output(results):
    C = np.zeros((N, N), np.float32)
    for c, r in enumerate(results):
        g, s = _core_gs(c)
        co = np.asarray(r["Cout"]).astype(np.float32).reshape(NT, P, QW)
        for te, (j, t) in enumerate(EMIT_PAIRS):
            b = 4 * j + g
            q = 2 * t + s
            C[P * b:P * b + P, QW * q:QW * q + QW] = co[te]
    return C


def kernel(A, B):
    nc = build_nc(MODE)
    in_maps = pack_inputs(A, B, MODE)
    res = bass_utils.run_bass_kernel_spmd(
        nc, in_maps, core_ids=list(range(NCORES)), trace=False)
    return unpack_output(res.results)
